# revision 13
# baseline (speedup 1.0000x reference)
"""Trainium2 Bass kernel for the ConvolutionalOverlap problem.

Reference computation (x: [2, 1, 256, 256] f32, w1/w2 scalar):
    out[b, i, h, w] = w1 * x[b, 0, h, w - (i+1)//2] + w2 * x[b, 0, h, w + (i+2)//2]
    (terms outside [0, W) are zero), out shape [2, 256, 256, 256].

Strategy (pure SPMD across 8 cores, identical program, different data):
  - Flatten (b, h) into 512 rows; shard 64 rows per core.
  - On each core, duplicate the 64 rows onto both SBUF partition halves:
    partitions 0..63 compute output columns w in [0, 128) and hold
    x zero-padded by 128 on the left; partitions 64..127 compute
    w in [128, 256) and hold x unshifted (zero-padded on the right).
    With that per-half staging, one free-dim access pattern serves all
    128 partitions, and the zero padding implements the boundary masks.
  - out[ch] = shift(x*w1, s1(ch)) + shift(x*w2, -s2(ch)).  s1/s2 are
    affine in the channel pair index, so one DVE scalar_tensor_tensor
    instruction per (group, parity) computes a whole channel group:
    out = (xp_shift1 * w1) + x2p_shift2, where x2p = w2*xp is staged once.
  - Channels are split into graduated groups [16,32,48,64,96]; each group
    gets one output DMA, alternating between the two HWDGE rings (SP/ACT),
    so the first DMA launches ~7 us in and the queue never starves.

Per core: ~0.2 MB in, 16 MB out -> DMA-write-bound.  Measured steady-state
on trn2: ~47.4 us per 16 MB output (354 GB/s/core, ~99% of the HBM-per-
NeuronCore write limit); single-shot incl. ramp ~58 us (CoreSim estimate).
"""

import sys

import numpy as np

if "/opt/trn_rl_repo" not in sys.path:
    sys.path.insert(0, "/opt/trn_rl_repo")

import concourse.bass as bass
import concourse.mybir as mybir
from concourse.ap import AP

F32 = mybir.dt.float32
P = 128          # SBUF partitions
W = 256          # spatial width == number of output channels
WH = W // 2      # output columns per partition half
XW = 388         # padded x width: j in [0, 384); cols 384/385 hold w1/w2
ROWS = 512       # B * H
NCORES = 8
RPC = ROWS // NCORES  # rows per core (64)
# Channel group sizes (sum 256).  Graduated so the first output DMA
# launches early (short ramp) while later, larger groups keep the DMA
# queue fed; DVE produces channels faster than DMA drains them, so the
# pipeline is DMA-bound after the first group.  1 in-DMA + len(GROUPS)
# out-DMAs must stay <= 8 (8 DMAHW sem lanes; a 9th DMA wraps onto lane 0
# adding a 2nd sync-wait, which this walrus codegen path rejects).
GROUPS = [16, 32, 48, 64, 96]

_nc_cache = None


def _sub(tile_ap, off, dims):
    """AP over `tile_ap`'s tensor: all 128 partitions, custom free dims."""
    if not isinstance(tile_ap, AP):
        tile_ap = tile_ap[:]
    part = list(tile_ap.ap)[0]
    return AP(
        tile_ap.tensor,
        tile_ap.offset + off,
        [list(part)] + [list(d) for d in dims],
    )


def build_nc():
    """Raw Bass (no TileContext): explicit sems, <=1 sync-wait per
    instruction (this walrus codegen path rejects multi-wait instructions,
    including Tile's tail drain)."""
    nc = bass.Bass(trn_type="TRN2")
    xp = nc.dram_tensor("xp", [P, XW], F32, kind="ExternalInput")
    out = nc.dram_tensor("out", [P, W * WH], F32, kind="ExternalOutput")

    from contextlib import ExitStack

    with ExitStack() as ctx:
        Xp = ctx.enter_context(nc.sbuf_tensor("Xp", [P, XW], F32))
        X2 = ctx.enter_context(nc.sbuf_tensor("X2", [P, 384], F32))
        Os = [
            ctx.enter_context(nc.sbuf_tensor(f"O{g}", [P, n * WH], F32))
            for g, n in enumerate(GROUPS)
        ]
        sem_in = ctx.enter_context(nc.semaphore("sem_in"))
        sem_x2 = ctx.enter_context(nc.semaphore("sem_x2"))
        sem_dve = ctx.enter_context(nc.semaphore("sem_dve"))
        sem_out = ctx.enter_context(nc.semaphore("sem_out"))

        # SP: load the packed input (x rows, padded + duplicated, w1/w2).
        nc.sync.dma_start(out=Xp[:], in_=xp[:]).then_inc(sem_in, 16)

        W1 = Xp[:, 384:385]
        W2 = Xp[:, 385:386]

        # DVE: X2 = w2 * x (padded); w1 is fused into the main op below.
        nc.vector.wait_ge(sem_in, 16)
        nc.vector.tensor_scalar_mul(X2[:], Xp[:, 0:384], W2).then_inc(sem_x2, 1)
        # Same-engine RAW: DVE reads may overtake in-flight DVE writes, so
        # the first consumer of X2 must wait for the producer to retire.
        nc.vector.wait_ge(sem_x2, 1)

        c0 = 0
        for g, n in enumerate(GROUPS):
            O = Os[g]
            base = c0 // 2  # s1 for (pair 0, parity 0)
            pairs = n // 2
            # ch = c0 + 2*pair + parity
            # s1 = base + pair + parity, s2 = base + pair + 1
            # Walrus caps ScalarTensorTensor APs at 3-D, so emit one
            # instruction per parity: [partitions, pairs, w'].
            last = None
            for parity in range(2):
                in0 = _sub(Xp, 128 - base - parity, [(-1, pairs), (1, WH)])
                in1 = _sub(X2, 129 + base, [(1, pairs), (1, WH)])
                o = _sub(O, parity * WH, [(2 * WH, pairs), (1, WH)])
                last = nc.vector.scalar_tensor_tensor(
                    o, in0, W1, in1,
                    mybir.AluOpType.mult, mybir.AluOpType.add,
                )
            last.then_inc(sem_dve, 1)
            c0 += n

        # Out DMAs alternate between the two HWDGE rings (SP / ACT).
        c0 = 0
        for g, n in enumerate(GROUPS):
            eng = nc.sync if g % 2 == 0 else nc.scalar
            eng.wait_ge(sem_dve, g + 1)
            eng.dma_start(
                out=out[:, c0 * WH:(c0 + n) * WH], in_=Os[g][:]
            ).then_inc(sem_out, 16)
            c0 += n

        # Each issuing engine waits for all out-DMA completions so the
        # NEFF doesn't finish with DMAs in flight.
        nc.sync.wait_ge(sem_out, 16 * len(GROUPS))
        nc.scalar.wait_ge(sem_out, 16 * len(GROUPS))
    return nc


def get_nc():
    global _nc_cache
    if _nc_cache is None:
        _nc_cache = build_nc()
    return _nc_cache


def prep_in_maps(x, w1, w2):
    """Shard + stage inputs for the 8 cores (host-side data movement only)."""
    x2 = np.ascontiguousarray(np.asarray(x, dtype=np.float32)[:, 0]).reshape(
        ROWS, W
    )
    w1v = np.float32(np.asarray(w1).reshape(-1)[0])
    w2v = np.float32(np.asarray(w2).reshape(-1)[0])
    in_maps = []
    for c in range(NCORES):
        rows = x2[c * RPC:(c + 1) * RPC]  # [64, 256]
        xp = np.zeros((P, XW), dtype=np.float32)
        xp[:RPC, 128:128 + W] = rows      # half 0: columns w in [0, 128)
        xp[RPC:, 0:W] = rows              # half 1: columns w in [128, 256)
        xp[:, 384] = w1v
        xp[:, 385] = w2v
        in_maps.append({"xp": xp})
    return in_maps


def gather(outs):
    """Reassemble per-core [128, 256*128] outputs into [2, 256, 256, 256]."""
    parts = []
    for oc in outs:
        oc = np.asarray(oc).reshape(2, RPC, W, WH)  # [whalf, row, ch, w']
        parts.append(oc.transpose(1, 2, 0, 3).reshape(RPC, W, W))
    out_rows = np.concatenate(parts, axis=0)        # [512 rows, ch, w]
    return np.ascontiguousarray(
        out_rows.reshape(2, 256, W, W).transpose(0, 2, 1, 3)
    )


def kernel(x, w1, w2, _run_kwargs=None):
    from concourse.bass_utils import run_bass_kernel_spmd

    nc = get_nc()
    in_maps = prep_in_maps(x, w1, w2)
    kwargs = _run_kwargs or {}
    res = run_bass_kernel_spmd(nc, in_maps, core_ids=list(range(NCORES)), **kwargs)
    out = gather([r["out"] for r in res.results])
    if kwargs:
        kernel.last_results = res
    return out
